# revision 34
# baseline (speedup 1.0000x reference)
"""Dilated (LongNet-style) attention kernel for 8 TRN2 NeuronCores.

Head-sharded SPMD design (core c owns heads {c, 8+c}), bf16 datapath:
  - Inputs stream in as bf16 (halves HBM traffic); all matmuls run bf16
    (enables fast-weight-load, which fp32r denies) with fp32 PSUM.
  - Per (branch, segment) job: scores are computed transposed ([key, query])
    in 512-col PSUM banks; the causal mask of the diagonal 128-block is
    pre-loaded into PSUM by a tiny identity x TRI matmul that also clears the
    bank (start=True), so no post-exp masking op is needed. exp runs on ACT
    into bf16 SBUF tiles; AV accumulates (with an appended ones column in V
    supplying the softmax denominators Z) into fp32 PSUM.
  - Branch merge = raw sum of exp-weighted AV and Z across branches
    (softmax-of-lse merge is algebraically A_tot/Z_tot). The accumulators
    accz0/accz1 hold [64 feats + Z] per head slot so each merge region is a
    single 65-partition DVE add into a zero-initialized buffer.
  - Projection matmuls are emitted as filler quanta *between* the attention
    jc-steps so the PE never idles while ACT computes exp.
  - Output redistribution is a position-split three-chunk AllToAll
    (positions 0..2048, 2048..3072, 3072..4096): jobs are ordered so each
    chunk's positions finish (merge + norm) as early as possible, chunks 0/1
    fly while later jobs compute (chunk 0's output projection rides as
    fillers inside the last job), and only chunk 2's flight is exposed.
    Stage DMAs ride SP (split per destination-half so each chunk launches
    right after its first norm block), gathers for chunks 0/1 ride the Pool
    queue right behind their collective (chunk 2's rides SP in shard halves
    so the last projection starts mid-copy), and outT ships via two wide
    DMAs per tail chunk.
"""

import sys

if "/opt/trn_rl_repo" not in sys.path:
    sys.path.insert(0, "/opt/trn_rl_repo")

import contextlib

import numpy as np
import ml_dtypes

import concourse.bacc as bacc
import concourse.bass as bass  # noqa: F401
import concourse.mybir as mybir
import concourse.tile as tile
from concourse import bass_utils

F32 = mybir.dt.float32
BF16 = mybir.dt.bfloat16
AF = mybir.ActivationFunctionType
BF = ml_dtypes.bfloat16

N_CORES = 8
E, L, H, D = 1024, 4096, 16, 64
KC = 8          # contraction chunks of 128 for the projections
PBP = 1024      # projection position block
NPP = L // PBP  # 4
PB = 512        # outproj position block (per core)
CW = 256        # a2a/outproj chunk width within each 512 block
G = 1024        # compressed segment length (all branches)
VBW = 65        # V_both per-chunk width (64 feats + ones col)

# constf columns: 0:2 bqk | 2:10 bo8 | 10:12 ws indicators
# constb columns: 0:128 tri | 128:256 eye | 256:272 ones


def _build():
    nc = bacc.Bacc("TRN2", target_bir_lowering=False, debug=False,
                   num_devices=N_CORES)

    qT = nc.dram_tensor("qT", [NPP, 128, KC * PBP], BF16, kind="ExternalInput")
    kT = nc.dram_tensor("kT", [NPP, 128, KC * PBP], BF16, kind="ExternalInput")
    vT = nc.dram_tensor("vT", [NPP, 128, KC * PBP], BF16, kind="ExternalInput")
    w3 = nc.dram_tensor("w3", [128, 3 * KC * 128], BF16, kind="ExternalInput")
    wo = nc.dram_tensor("wo", [128, 8 * E], BF16, kind="ExternalInput")
    constf = nc.dram_tensor("constf", [128, 12], F32, kind="ExternalInput")
    constb = nc.dram_tensor("constb", [128, 272], BF16, kind="ExternalInput")
    ind2d = nc.dram_tensor("ind2d", [33, 128], BF16, kind="ExternalInput")

    outT = nc.dram_tensor("outT", [E, PB], BF16, kind="ExternalOutput")

    a2a_warm_in = nc.dram_tensor("a2a_warm_in", [8, 1, 64], BF16)
    a2a_warm_out = nc.dram_tensor("a2a_warm_out", [8, 1, 64], BF16)
    # position-split chunks: [0,2048) cw=256, [2048,3072) cw=128,
    # [3072,4096) cw=128   (per-destination-core column widths)
    CHUNKS = ((0, 256), (2048, 128), (3072, 128))
    a2a_in = [nc.dram_tensor(f"a2a_in{h}", [8, 128, cw], BF16)
              for h, (_, cw) in enumerate(CHUNKS)]
    a2a_out = [nc.dram_tensor(f"a2a_out{h}", [8, 128, cw], BF16)
               for h, (_, cw) in enumerate(CHUNKS)]

    def _emit(tc, ctx):
        pin = ctx.enter_context(tc.tile_pool(name="pin", bufs=5))
        persist = ctx.enter_context(tc.tile_pool(name="persist", bufs=1))
        vpool = ctx.enter_context(tc.tile_pool(name="vpool", bufs=2))
        epool = ctx.enter_context(tc.tile_pool(name="epool", bufs=5))
        opool = ctx.enter_context(tc.tile_pool(name="opool", bufs=6))
        psS = ctx.enter_context(tc.tile_pool(name="psS", bufs=2, space="PSUM"))
        psO = ctx.enter_context(tc.tile_pool(name="psO", bufs=2, space="PSUM"))

        # ---- persistent tiles ----
        w3_sb = persist.tile([128, 3 * KC * 128], BF16, tag="w3")
        wo_sb = persist.tile([128, 8 * E], BF16, tag="wo")
        cf = persist.tile([128, 12], F32, tag="cf")
        cb = persist.tile([128, 272], BF16, tag="cb")
        ind_sb = persist.tile([33, 128], BF16, tag="ind")

        QT = persist.tile([128, L], BF16, tag="QT")
        KT = persist.tile([128, L], BF16, tag="KT")
        VT = persist.tile([128, L], BF16, tag="VT")
        QT2 = persist.tile([128, G], BF16, tag="QT2")
        KT2 = persist.tile([128, G], BF16, tag="KT2")
        VT2 = persist.tile([128, G], BF16, tag="VT2")
        # per-slot accumulators: rows 0:64 = features, row 64 = Z
        accz0 = persist.tile([VBW, L], F32, tag="accz0")
        accz1 = persist.tile([VBW, L], F32, tag="accz1")
        accz = [accz0, accz1]
        zzr = persist.tile([33, L], BF16, tag="zzr")
        accb = persist.tile([128, L], BF16, tag="accb")
        mgr = [persist.tile([128, 8 * cw], BF16, tag="mgr", name=f"mgr{h}")
               for h, (_, cw) in enumerate(CHUNKS)]

        TRI = cb[:, 0:128]
        ONES16 = cb[:, 256:272]

        def eye_s(slot):
            return cb[slot * 64:(slot + 1) * 64,
                      128 + slot * 64:128 + (slot + 1) * 64]

        # ---- weights first (split per stream; 2KB lines), then consts ----
        for soff, eng in ((0, nc.sync), (1, nc.gpsimd), (2, nc.scalar)):
            for whh in range(2):
                wc0 = soff * 1024 + whh * 512
                eng.dma_start(w3_sb[:, wc0:wc0 + 512], w3[:, wc0:wc0 + 512])
        nc.gpsimd.dma_start(cf[:], constf[:])
        nc.gpsimd.dma_start(cb[:], constb[:])
        nc.gpsimd.dma_start(ind_sb[:], ind2d[:])
        for i in range(4):
            nc.scalar.dma_start(wo_sb[:, i * 2048:(i + 1) * 2048],
                                wo[:, i * 2048:(i + 1) * 2048])

        # zero accumulators (merges are pure adds) and the zzr pad rows
        nc.vector.memset(accz0[:], 0.0)
        nc.vector.memset(accz1[:], 0.0)
        nc.vector.memset(zzr[:], 0.0)

        # warm the ACT exp table early
        wtile = opool.tile([1, 16], BF16, tag="warm")
        nc.scalar.activation(wtile[:], ONES16[0:1, 0:16], AF.Exp)

        # tiny dummy collective absorbs first-collective setup cost;
        # emitted before the xin trigger flood so the gpsimd queue reaches it
        for rr in range(8):
            nc.scalar.dma_start(a2a_warm_in[rr][0:1, 0:16], ONES16[0:1, 0:16])
        nc.gpsimd.collective_compute(
            "AllToAll", mybir.AluOpType.bypass,
            replica_groups=[list(range(8))],
            ins=[a2a_warm_in[:]], outs=[a2a_warm_out[:]],
        )

        # ---- stream all input position blocks up-front ----
        streams = (("k", kT, KT, 0), ("v", vT, VT, 1), ("q", qT, QT, 2))
        xin_tiles = {}
        qengs = (nc.sync, nc.gpsimd)
        qi = 0
        HW_ = KC * PBP // 2
        for pb in range(NPP):
            for name, x_d, _, _ in streams:
                xin = pin.tile([128, KC * PBP], BF16, tag="xin")
                for hh in range(2):
                    qengs[qi % 2].dma_start(
                        xin[:, hh * HW_:(hh + 1) * HW_],
                        x_d[pb][:, hh * HW_:(hh + 1) * HW_],
                    )
                    qi += 1
                xin_tiles[(pb, name)] = xin

        # ---- projection work quanta ----
        def proj_quantum(pb, sname, half):
            xin = xin_tiles[(pb, sname)]
            _, _, dst, soff = next(s for s in streams if s[0] == sname)
            pt = psS.tile([128, 512], F32, tag="ps")
            c0 = half * 512
            for kc in range(KC):
                nc.tensor.matmul(
                    pt[:, 0:512],
                    w3_sb[:, soff * 1024 + kc * 128:soff * 1024 + (kc + 1) * 128],
                    xin[:, kc * PBP + c0:kc * PBP + c0 + 512],
                    start=(kc == 0), stop=(kc == KC - 1),
                )
            dslice = dst[:, pb * PBP + c0:pb * PBP + c0 + 512]
            if sname == "q":
                nc.vector.tensor_scalar_add(dslice, pt[:, 0:512], cf[:, 0:1])
            elif sname == "k":
                nc.vector.tensor_scalar_add(dslice, pt[:, 0:512], cf[:, 1:2])
            else:
                nc.vector.tensor_copy(dslice, pt[:, 0:512])

        def quanta_for_pb(pb):
            return [(lambda p=pb, s=s, h=h: proj_quantum(p, s, h))
                    for s in ("k", "v", "q") for h in (0, 1)]

        # ---- branch-2 dilation-compressed copies (per pb chunk) ----
        def b2_pair(pb, src, dst):
            for slot in range(2):
                p0 = 64 * slot
                o0 = 2 * slot
                dc = dst[p0:p0 + 64, pb * 256:(pb + 1) * 256]
                s0 = pb * PBP + o0
                nc.vector.tensor_scalar_mul(
                    dc,
                    src[p0:p0 + 64, s0:s0 + 4 * 255 + 1:4],
                    cf[p0:p0 + 64, 10:11],
                )
                nc.vector.scalar_tensor_tensor(
                    dc,
                    src[p0:p0 + 64, s0 + 1:s0 + 1 + 4 * 255 + 1:4],
                    cf[p0:p0 + 64, 11:12], dc,
                    mybir.AluOpType.mult, mybir.AluOpType.add,
                )

        def b2_v(pb):
            b2_pair(pb, VT, VT2)

        def b2_kq(pb):
            b2_pair(pb, KT, KT2)
            b2_pair(pb, QT, QT2)

        # ---- normalization: recip Z rows in place, broadcast via matmul ----
        def norm_recip(c0, w):
            with nc.allow_low_precision(reason="softmax denom reciprocal"):
                nc.vector.reciprocal(zzr[0:1, c0:c0 + w],
                                     accz0[64:65, c0:c0 + w])
                nc.vector.reciprocal(zzr[32:33, c0:c0 + w],
                                     accz1[64:65, c0:c0 + w])

        def norm_block35():
            # cols 3584..4096 (upper half of block 3)
            rb = psS.tile([128, 1024], F32, tag="ps")
            nc.tensor.matmul(rb[:, 0:512], ind_sb[:], zzr[0:33, 3584:4096],
                             start=True, stop=True)
            with nc.allow_low_precision(reason="bf16 a2a payload"):
                nc.vector.tensor_mul(accb[0:64, 3584:4096],
                                     accz0[0:64, 3584:4096], rb[0:64, 0:512])
                nc.vector.tensor_mul(accb[64:128, 3584:4096],
                                     accz1[0:64, 3584:4096], rb[64:128, 0:512])

        def norm_block(nb, w=1024):
            # rb = 1/Z broadcast to all 128 partitions via indicator matmul
            c0 = nb * 1024
            rb = psS.tile([128, 1024], F32, tag="ps")
            for hh in range(0, w, 512):
                nc.tensor.matmul(
                    rb[:, hh:hh + 512], ind_sb[:],
                    zzr[0:33, c0 + hh:c0 + hh + 512],
                    start=True, stop=True,
                )
            with nc.allow_low_precision(reason="bf16 a2a payload"):
                nc.vector.tensor_mul(
                    accb[0:64, c0:c0 + w],
                    accz0[0:64, c0:c0 + w], rb[0:64, 0:w])
                nc.vector.tensor_mul(
                    accb[64:128, c0:c0 + w],
                    accz1[0:64, c0:c0 + w], rb[64:128, 0:w])

        # ---- a2a staging / launch / gather (position-split chunks) ----
        # stage rides the DVE queue (fires right after the norm muls that
        # produce accb); gather rides the Pool queue (blocked during the
        # collective's flight, so it fires exactly at flight end). This
        # keeps the tile scheduler from interleaving them with outT DMAs.
        def a2a_stage(h, part=None):
            p0, cw = CHUNKS[h]
            r0, r1 = 0, 8
            if part is not None:
                r0, r1 = (0, 4) if part == 0 else (4, 8)
            nc.sync.dma_start(
                a2a_in[h][r0:r1].rearrange("r p c -> p r c"),
                accb[:, p0 + r0 * cw:p0 + r1 * cw].rearrange(
                    "p (r c) -> p r c", r=r1 - r0),
            )

        def a2a_launch(h, gather=True):
            nc.gpsimd.collective_compute(
                "AllToAll", mybir.AluOpType.bypass,
                replica_groups=[list(range(8))],
                ins=[a2a_in[h][:]], outs=[a2a_out[h][:]],
            )
            if gather:
                nc.gpsimd.dma_start(
                    mgr[h][:].rearrange("p (s c) -> p s c", s=8),
                    a2a_out[h][:].rearrange("s p c -> p s c"),
                )

        def a2a_gather_sp(h):
            # two halves: the first four shards land first so the output
            # projection's ec 0..3 matmuls can start during the second copy
            for s0, s1 in ((0, 4), (4, 8)):
                nc.sync.dma_start(
                    mgr[h][:].rearrange("p (s c) -> p s c", s=8)[:, s0:s1],
                    a2a_out[h][s0:s1].rearrange("s p c -> p s c"),
                )

        # outT column ranges per chunk
        OC0 = (0, 256, 384)

        # wide staging tiles for the tail chunks: all 8 ob blocks land
        # side by side so one DMA ships the whole chunk
        osb_all = [persist.tile([128, 8 * CHUNKS[h][1]], BF16, tag="osba",
                                name=f"osba{h}") for h in (1, 2)]

        # ---- output projection quantum: one 128-row output block ----
        def oq(h, ob):
            cw = CHUNKS[h][1]
            pt = psS.tile([128, 512], F32, tag="ps")
            for ec in range(KC):
                nc.tensor.matmul(
                    pt[:, 0:cw],
                    wo_sb[:, ec * E + ob * 128:ec * E + (ob + 1) * 128],
                    mgr[h][:, ec * cw:(ec + 1) * cw],
                    start=(ec == 0), stop=(ec == KC - 1),
                )
            if h == 0:
                osb = opool.tile([128, 256], BF16, tag="osb")
                # DVE during the last job (ACT is the per-job bottleneck)
                nc.vector.tensor_scalar_add(osb[:, 0:cw], pt[:, 0:cw],
                                            cf[:, 2 + ob:3 + ob])
                nc.sync.dma_start(
                    outT[ob * 128:(ob + 1) * 128, OC0[0]:OC0[0] + cw],
                    osb[:, 0:cw])
            else:
                # tail chunks: both ACT and DVE are idle — alternate so the
                # bias adds drain in parallel; ship once per chunk
                dst = osb_all[h - 1][:, ob * cw:(ob + 1) * cw]
                if ob % 2 == 0:
                    nc.scalar.activation(dst, pt[:, 0:cw], AF.Identity,
                                         bias=cf[:, 2 + ob:3 + ob])
                else:
                    nc.vector.tensor_scalar_add(dst, pt[:, 0:cw],
                                                cf[:, 2 + ob:3 + ob])

        def out_ship(h, half):
            cw = CHUNKS[h][1]
            o0, o1 = (0, 4) if half == 0 else (4, 8)
            nc.sync.dma_start(
                outT[o0 * 128:o1 * 128, OC0[h]:OC0[h] + cw].rearrange(
                    "(ob p) c -> p ob c", p=128),
                osb_all[h - 1][:, o0 * cw:o1 * cw].rearrange(
                    "p (ob c) -> p ob c", ob=o1 - o0))

        # ---- K/Q slicing per branch ----
        def kq_slice(br, seg, slot, t, lo, size):
            if br == 0:
                base = 1024 * seg + lo
                return t[slot * 64:(slot + 1) * 64, base:base + size]
            if br == 1:
                base = 2048 * seg + 2 * lo + slot
                return t[slot * 64:(slot + 1) * 64,
                         base:base + 2 * size - slot:2]
            return t[slot * 64:(slot + 1) * 64, lo:lo + size]

        # ---- V_both prep: PE transposes; copies ride ACT (idle at job
        # tails) so the merge-laden DVE queue never gates the next job
        def vb_prep(br, seg, vb=None, jcs=range(8)):
            if vb is None:
                vb = vpool.tile([128, 2 * 8 * VBW], BF16, tag="vb")
                nc.scalar.copy(vb[:, 64::VBW], ONES16)
            for jc in jcs:
                if br == 0:
                    tp = psS.tile([128, 128], BF16, tag="ps")
                    src = VT[:, 1024 * seg + 128 * jc:1024 * seg + 128 * (jc + 1)]
                    nc.tensor.transpose(tp[:, 0:128], src, cb[:, 128:256])
                    dst = vb[:].rearrange(
                        "p (s jj t) -> p s jj t", s=2, jj=8
                    )[:, :, jc, 0:64]
                    srcp = tp[:, 0:128].rearrange("p (s r) -> p s r", s=2)
                    nc.scalar.copy(dst, srcp)
                else:
                    for slot in range(2):
                        tp = psS.tile([128, 128], BF16, tag="ps")
                        if br == 1:
                            base = 2048 * seg + 256 * jc + slot
                            src = VT[slot * 64:(slot + 1) * 64,
                                     base:base + 256 - slot:2]
                        else:
                            src = VT2[slot * 64:(slot + 1) * 64,
                                      128 * jc:128 * (jc + 1)]
                        nc.tensor.transpose(tp[:, 0:64], src, eye_s(slot))
                        nc.scalar.copy(
                            vb[:, slot * 8 * VBW + jc * VBW:
                               slot * 8 * VBW + jc * VBW + 64],
                            tp[:, 0:64],
                        )
            return vb

        # ---- one (branch, segment) job ----
        def job(br, seg, fillers, vb=None, defer_final=False):
            kt_src = KT2 if br == 2 else KT
            qt_src = QT2 if br == 2 else QT
            fillers = list(fillers)
            if vb is None:
                vb = vb_prep(br, seg)

            o_ps_a = psO.tile([128, 1024], F32, tag="o")
            o_ps_b = psO.tile([128, 1024], F32, tag="o")
            o_ps = [o_ps_a, o_ps_b]

            def merge(r0, r1):
                # scatter o_ps[slot] rows [feats|Z] region [r0:r1] into accz
                for slot in range(2):
                    op = o_ps[slot]
                    az = accz[slot]
                    w = r1 - r0
                    if br == 0:
                        d0 = 1024 * seg + r0
                        dst = az[0:VBW, d0:d0 + w]
                        nc.vector.tensor_add(dst, dst, op[0:VBW, r0:r1])
                    elif br == 1:
                        d0 = 2048 * seg + 2 * r0 + slot
                        d1 = d0 + 2 * (w - 1) + 1
                        dst = az[0:VBW, d0:d1:2]
                        nc.vector.tensor_add(dst, dst, op[0:VBW, r0:r1])
                    else:
                        o0 = 2 * slot
                        for dd in range(2):
                            d0 = 4 * r0 + o0 + dd
                            d1 = d0 + 4 * (w - 1) + 1
                            dst = az[0:VBW, d0:d1:4]
                            nc.vector.scalar_tensor_tensor(
                                dst, op[0:VBW, r0:r1],
                                cf[0:VBW, 10 + dd:11 + dd],
                                dst, mybir.AluOpType.mult, mybir.AluOpType.add,
                            )

            def emit_avs(jc, es):
                c0 = 128 * jc
                for slot in range(2):
                    e = es[slot]
                    vbs = vb[:, slot * 8 * VBW + jc * VBW:
                             slot * 8 * VBW + (jc + 1) * VBW]
                    if c0 < 512:
                        nc.tensor.matmul(
                            o_ps[slot][0:VBW, c0:512], vbs, e[:, c0:512],
                            start=(jc == 0), stop=(jc == 3),
                        )
                        nc.tensor.matmul(
                            o_ps[slot][0:VBW, 512:1024], vbs, e[:, 512:1024],
                            start=(jc == 0), stop=(jc == 7),
                        )
                    else:
                        nc.tensor.matmul(
                            o_ps[slot][0:VBW, c0:1024], vbs, e[:, c0:1024],
                            start=(jc == 0), stop=(jc == 7),
                        )

            prev_es = None
            for jc in range(8):
                c0 = 128 * jc
                es = []
                for slot in range(2):
                    s = psS.tile([128, 1024], F32, tag="ps")
                    lhs = kq_slice(br, seg, slot, kt_src, c0, 128)
                    # causal-mask bias first: clears the bank (start=True),
                    # writes -100 upper-triangle into the diagonal block.
                    nc.tensor.matmul(
                        s[:, c0:c0 + 128], cb[:, 128:256], TRI,
                        start=True, stop=False,
                    )
                    if c0 < 512:
                        nc.tensor.matmul(
                            s[:, c0:512], lhs,
                            kq_slice(br, seg, slot, qt_src, c0, 512 - c0),
                            start=False, stop=True,
                            tile_position=(slot * 64, 0),
                        )
                        nc.tensor.matmul(
                            s[:, 512:1024], lhs,
                            kq_slice(br, seg, slot, qt_src, 512, 512),
                            start=True, stop=True,
                            tile_position=(slot * 64, 0),
                        )
                    else:
                        nc.tensor.matmul(
                            s[:, c0:1024], lhs,
                            kq_slice(br, seg, slot, qt_src, c0, 1024 - c0),
                            start=False, stop=True,
                            tile_position=(slot * 64, 0),
                        )
                    e = epool.tile([128, 1024], BF16, tag="e")
                    nc.scalar.activation(e[:, c0:1024], s[:, c0:1024], AF.Exp)
                    es.append(e)

                if fillers:
                    f = fillers.pop(0)
                    if f is not None:
                        f()

                if prev_es is not None:
                    emit_avs(jc - 1, prev_es)
                    if jc - 1 == 3:
                        merge(0, 512)
                prev_es = es
            emit_avs(7, prev_es)
            if defer_final:
                ret = lambda: merge(512, 1024)
            else:
                merge(512, 1024)
                ret = None
            for f in fillers:
                if f is not None:
                    f()
            return ret

        # ================= emission order =================
        # jobs ordered so positions 0..2048 finish (merge + norm) early:
        # chunk-0 a2a flies over jobs 5-6; chunk-1 (positions 2048..3072,
        # complete once br1 seg1's first-half merge lands) flies over job 7,
        # whose fillers run chunk-0's output projection; chunk-2 (3072..4096)
        # is the only exposed flight.
        for q in quanta_for_pb(0):
            q()
        b2_v(0)
        b2_kq(0)
        job(0, 0, quanta_for_pb(1) + [lambda: b2_v(1), lambda: b2_kq(1)])
        job(0, 1, quanta_for_pb(2) + [lambda: b2_v(2), lambda: b2_kq(2)])
        job(1, 0, quanta_for_pb(3) + [lambda: b2_v(3)])
        b2_kq(3)
        vbh = [None]

        def vbp(br, seg, jcs):
            def f():
                vbh[0] = vb_prep(br, seg, vb=vbh[0], jcs=jcs)
            return f

        m20 = job(2, 0, [None, None, None, None, None,
                         lambda: norm_recip(0, 2048),
                         vbp(0, 2, range(0, 4)), vbp(0, 2, range(4, 8))],
                  defer_final=True)
        vb02, vbh[0] = vbh[0], None
        job(0, 2, [lambda: (norm_block(0), a2a_stage(0, part=0)),
                   lambda: (norm_block(1), a2a_stage(0, part=1),
                            a2a_launch(0)),
                   m20, None, None, None, None, None], vb=vb02)
        m11 = job(1, 1, [None, None, None, None, None,
                         lambda: norm_recip(2048, 1024),
                         vbp(0, 3, range(0, 4)), vbp(0, 3, range(4, 8))],
                  defer_final=True)
        vb03, vbh[0] = vbh[0], None
        job(0, 3, [lambda: norm_block(2),
                   lambda: (a2a_stage(1), a2a_launch(1)), m11,
                   (lambda: oq(0, 0)), (lambda: oq(0, 1)),
                   (lambda: norm_recip(3072, 512)),
                   (lambda: (norm_block(3, 512), a2a_stage(2, part=0))),
                   (lambda: oq(0, 2))], vb=vb03)
        for b in range(3, 8):
            oq(0, b)

        # ---- last quarter tail: second 512, chunk-2 launch, projections ----
        norm_recip(3584, 512)
        norm_block35()
        a2a_stage(2, part=1)
        a2a_launch(2, gather=False)
        for b in range(8):
            oq(1, b)
        out_ship(1, 0)
        out_ship(1, 1)
        a2a_gather_sp(2)
        for b in range(8):
            oq(2, b)
        out_ship(2, 0)
        out_ship(2, 1)

    with tile.TileContext(nc) as tc, contextlib.ExitStack() as ctx:
        _emit(tc, ctx)

    nc.compile()
    return nc


_NC_CACHE = {}


def _get_nc():
    if "nc" not in _NC_CACHE:
        _NC_CACHE["nc"] = _build()
    return _NC_CACHE["nc"]


def _prep_inputs(query, key, value, Wq, bq, Wk, bk, Wv, bv, Wo, bo):
    """Host-side layout prep. Returns in_maps for the 8 cores."""
    def _tilein(x):
        # [pb, p, kc*1024] with 16KB-contiguous per-partition lines
        xT = np.ascontiguousarray(x[0].T).astype(BF)   # (E, L)
        xp = xT.reshape(KC, 128, NPP, PBP).transpose(2, 1, 0, 3)
        return np.ascontiguousarray(xp.reshape(NPP, 128, KC * PBP))

    qT = _tilein(query)
    kT = _tilein(key)
    vT = _tilein(value)

    WqT = np.ascontiguousarray(Wq.T) * np.float32(0.125)
    WkT = np.ascontiguousarray(Wk.T)
    WvT = np.ascontiguousarray(Wv.T)

    # permuted Wo.T rows to match a2a arriving-feature order
    perm = np.concatenate(
        [np.r_[64 * s:64 * s + 64, 512 + 64 * s:512 + 64 * s + 64]
         for s in range(8)]
    )
    WoT = np.ascontiguousarray(Wo.T)[perm]            # (E e', E o)
    wo_pack = np.zeros((128, 8 * E), np.float32)
    for ec in range(8):
        wo_pack[:, ec * E:(ec + 1) * E] = WoT[ec * 128:(ec + 1) * 128]
    wo_pack = wo_pack.astype(BF)

    bo_eff = (bo + bv @ Wo.T).astype(np.float32)
    bo8 = bo_eff.reshape(8, 128).T.copy()             # [p, ob]

    IND = np.zeros((33, 128), np.float32)
    IND[0, 0:64] = 1.0
    IND[32, 64:128] = 1.0
    # [k, q] layout: mask where q < k
    TRI = np.zeros((128, 128), np.float32)
    ki, qi = np.meshgrid(np.arange(128), np.arange(128), indexing="ij")
    TRI[qi < ki] = -100.0
    EYE = np.eye(128, dtype=np.float32)

    constb = np.zeros((128, 272), np.float32)
    constb[:, 0:128] = TRI
    constb[:, 128:256] = EYE
    constb[:, 256:272] = 1.0
    constb = constb.astype(BF)

    in_maps = []
    for c in range(8):
        fa = np.r_[64 * c:64 * c + 64]
        fb = np.r_[512 + 64 * c:512 + 64 * c + 64]
        sel = np.concatenate([fa, fb])
        w3 = np.zeros((128, 3 * KC * 128), np.float32)
        for soff, WT in ((0, WkT), (1, WvT), (2, WqT)):
            for kc in range(KC):
                w3[:, soff * 1024 + kc * 128:soff * 1024 + (kc + 1) * 128] = \
                    WT[kc * 128:(kc + 1) * 128][:, sel]
        constf = np.zeros((128, 12), np.float32)
        constf[:, 0] = bq[sel] * np.float32(0.125)
        constf[:, 1] = bk[sel]
        constf[:, 2:10] = bo8
        constf[:, 10 + c // 4] = 1.0   # branch-2 offset indicator
        in_maps.append({
            "qT": qT, "kT": kT, "vT": vT,
            "w3": w3.astype(BF),
            "wo": wo_pack,
            "constf": constf,
            "constb": constb,
            "ind2d": IND.astype(BF),
        })
    return in_maps


def kernel(query, key, value, Wq, bq, Wk, bk, Wv, bv, Wo, bo,
           _trace=False, _result_holder=None):
    args = [np.asarray(a, np.float32) for a in
            (query, key, value, Wq, bq, Wk, bk, Wv, bv, Wo, bo)]
    nc = _get_nc()
    in_maps = _prep_inputs(*args)
    res = bass_utils.run_bass_kernel_spmd(
        nc, in_maps, core_ids=list(range(N_CORES)), trace=_trace
    )
    if _result_holder is not None:
        _result_holder.append(res)
    # core c holds positions [256c, 256c+256) (cols 0:256),
    # [2048+128c, 2048+128c+128) (cols 256:384) and
    # [3072+128c, 3072+128c+128) (cols 384:512)
    outT = np.zeros((E, L), np.float32)
    for c in range(N_CORES):
        r = res.results[c]["outT"]
        outT[:, 256 * c:256 * (c + 1)] = r[:, 0:256]
        outT[:, 2048 + 128 * c:2048 + 128 * (c + 1)] = r[:, 256:384]
        outT[:, 3072 + 128 * c:3072 + 128 * (c + 1)] = r[:, 384:512]
    return np.ascontiguousarray(outT.T).reshape(1, L, E)


# revision 41
# speedup vs baseline: 1.0077x; 1.0077x over previous
"""Dilated (LongNet-style) attention kernel for 8 TRN2 NeuronCores.

Head-sharded SPMD design (core c owns heads {c, 8+c}), bf16 datapath:
  - Inputs stream in as bf16 (halves HBM traffic); all matmuls run bf16
    (enables fast-weight-load, which fp32r denies) with fp32 PSUM.
  - Per (branch, segment) job: scores are computed transposed ([key, query])
    in 512-col PSUM banks; the causal mask of the diagonal 128-block is
    pre-loaded into PSUM by a tiny identity x TRI matmul that also clears the
    bank (start=True), so no post-exp masking op is needed. exp runs on ACT
    into bf16 SBUF tiles; AV accumulates (with an appended ones column in V
    supplying the softmax denominators Z) into fp32 PSUM.
  - Branch merge = raw sum of exp-weighted AV and Z across branches
    (softmax-of-lse merge is algebraically A_tot/Z_tot). The accumulators
    accz0/accz1 hold [64 feats + Z] per head slot so each merge region is a
    single 65-partition DVE add into a zero-initialized buffer.
  - Projection matmuls are emitted as filler quanta *between* the attention
    jc-steps so the PE never idles while ACT computes exp.
  - Output redistribution is a position-split three-chunk AllToAll
    (positions 0..2048, 2048..3072, 3072..4096): jobs are ordered so each
    chunk's positions finish (merge + norm) as early as possible, chunks 0/1
    fly while later jobs compute (chunk 0's output projection rides as
    fillers inside the last job), and only chunk 2's flight is exposed.
    Stage DMAs ride SP (split per destination-half so each chunk launches
    right after its first norm block), gathers for chunks 0/1 ride the Pool
    queue right behind their collective (chunk 2's rides SP in shard halves
    so the last projection starts mid-copy), and outT ships via two wide
    DMAs per tail chunk.
"""

import sys

if "/opt/trn_rl_repo" not in sys.path:
    sys.path.insert(0, "/opt/trn_rl_repo")

import contextlib

import numpy as np
import ml_dtypes

import concourse.bacc as bacc
import concourse.bass as bass  # noqa: F401
import concourse.mybir as mybir
import concourse.tile as tile
from concourse import bass_utils

F32 = mybir.dt.float32
BF16 = mybir.dt.bfloat16
AF = mybir.ActivationFunctionType
BF = ml_dtypes.bfloat16

N_CORES = 8
E, L, H, D = 1024, 4096, 16, 64
KC = 8          # contraction chunks of 128 for the projections
PBP = 1024      # projection position block
NPP = L // PBP  # 4
PB = 512        # outproj position block (per core)
CW = 256        # a2a/outproj chunk width within each 512 block
G = 1024        # compressed segment length (all branches)
VBW = 65        # V_both per-chunk width (64 feats + ones col)

# constf columns: 0:2 bqk | 2:10 bo8 | 10:12 ws indicators
# constb columns: 0:128 tri | 128:256 eye | 256:272 ones


def _build():
    nc = bacc.Bacc("TRN2", target_bir_lowering=False, debug=False,
                   num_devices=N_CORES)

    qT = nc.dram_tensor("qT", [NPP, 128, KC * PBP], BF16, kind="ExternalInput")
    kT = nc.dram_tensor("kT", [NPP, 128, KC * PBP], BF16, kind="ExternalInput")
    vT = nc.dram_tensor("vT", [NPP, 128, KC * PBP], BF16, kind="ExternalInput")
    w3 = nc.dram_tensor("w3", [128, 3 * KC * 128], BF16, kind="ExternalInput")
    wo = nc.dram_tensor("wo", [128, 8 * E], BF16, kind="ExternalInput")
    constf = nc.dram_tensor("constf", [128, 12], F32, kind="ExternalInput")
    constb = nc.dram_tensor("constb", [128, 272], BF16, kind="ExternalInput")
    ind2d = nc.dram_tensor("ind2d", [33, 128], BF16, kind="ExternalInput")

    outT = nc.dram_tensor("outT", [E, PB], BF16, kind="ExternalOutput")

    a2a_warm_in = nc.dram_tensor("a2a_warm_in", [8, 1, 64], BF16)
    a2a_warm_out = nc.dram_tensor("a2a_warm_out", [8, 1, 64], BF16)
    # position-split chunks: [0,2048) cw=256, [2048,3072) cw=128,
    # [3072,4096) cw=128   (per-destination-core column widths)
    CHUNKS = ((0, 256), (2048, 128), (3072, 128))
    a2a_in = [nc.dram_tensor(f"a2a_in{h}", [8, 128, cw], BF16)
              for h, (_, cw) in enumerate(CHUNKS)]
    a2a_out = [nc.dram_tensor(f"a2a_out{h}", [8, 128, cw], BF16)
               for h, (_, cw) in enumerate(CHUNKS)]

    def _emit(tc, ctx):
        pin = ctx.enter_context(tc.tile_pool(name="pin", bufs=5))
        persist = ctx.enter_context(tc.tile_pool(name="persist", bufs=1))
        vpool = ctx.enter_context(tc.tile_pool(name="vpool", bufs=2))
        epool = ctx.enter_context(tc.tile_pool(name="epool", bufs=5))
        opool = ctx.enter_context(tc.tile_pool(name="opool", bufs=6))
        psS = ctx.enter_context(tc.tile_pool(name="psS", bufs=2, space="PSUM"))
        psO = ctx.enter_context(tc.tile_pool(name="psO", bufs=2, space="PSUM"))

        # ---- persistent tiles ----
        w3_sb = persist.tile([128, 3 * KC * 128], BF16, tag="w3")
        wo_sb = persist.tile([128, 8 * E], BF16, tag="wo")
        cf = persist.tile([128, 12], F32, tag="cf")
        cb = persist.tile([128, 272], BF16, tag="cb")
        ind_sb = persist.tile([33, 128], BF16, tag="ind")

        QT = persist.tile([128, L], BF16, tag="QT")
        KT = persist.tile([128, L], BF16, tag="KT")
        VT = persist.tile([128, L], BF16, tag="VT")
        QT2 = persist.tile([128, G], BF16, tag="QT2")
        KT2 = persist.tile([128, G], BF16, tag="KT2")
        VT2 = persist.tile([128, G], BF16, tag="VT2")
        # per-slot accumulators: rows 0:64 = features, row 64 = Z
        accz0 = persist.tile([VBW, L], F32, tag="accz0")
        accz1 = persist.tile([VBW, L], F32, tag="accz1")
        accz = [accz0, accz1]
        zzr = persist.tile([33, L], BF16, tag="zzr")
        accb = persist.tile([128, L], BF16, tag="accb")
        mgr = [persist.tile([128, 8 * cw], BF16, tag="mgr", name=f"mgr{h}")
               for h, (_, cw) in enumerate(CHUNKS)]

        TRI = cb[:, 0:128]
        ONES16 = cb[:, 256:272]

        def eye_s(slot):
            return cb[slot * 64:(slot + 1) * 64,
                      128 + slot * 64:128 + (slot + 1) * 64]

        # ---- weights first (split per stream; 2KB lines), then consts ----
        for soff, eng in ((0, nc.sync), (1, nc.gpsimd), (2, nc.scalar)):
            for whh in range(2):
                wc0 = soff * 1024 + whh * 512
                eng.dma_start(w3_sb[:, wc0:wc0 + 512], w3[:, wc0:wc0 + 512])
        nc.gpsimd.dma_start(cf[:], constf[:])
        nc.gpsimd.dma_start(cb[:], constb[:])
        nc.gpsimd.dma_start(ind_sb[:], ind2d[:])
        for i in range(4):
            nc.scalar.dma_start(wo_sb[:, i * 2048:(i + 1) * 2048],
                                wo[:, i * 2048:(i + 1) * 2048])

        # zero accumulators (merges are pure adds) and the zzr pad rows
        nc.vector.memset(accz0[:], 0.0)
        nc.vector.memset(accz1[:], 0.0)
        nc.vector.memset(zzr[:], 0.0)

        # warm the ACT exp table early
        wtile = opool.tile([1, 16], BF16, tag="warm")
        nc.scalar.activation(wtile[:], ONES16[0:1, 0:16], AF.Exp)

        # tiny dummy collective absorbs first-collective setup cost;
        # emitted before the xin trigger flood so the gpsimd queue reaches it
        for rr in range(8):
            nc.scalar.dma_start(a2a_warm_in[rr][0:1, 0:16], ONES16[0:1, 0:16])
        nc.gpsimd.collective_compute(
            "AllToAll", mybir.AluOpType.bypass,
            replica_groups=[list(range(8))],
            ins=[a2a_warm_in[:]], outs=[a2a_warm_out[:]],
        )

        # ---- stream all input position blocks up-front ----
        streams = (("k", kT, KT, 0), ("v", vT, VT, 1), ("q", qT, QT, 2))
        xin_tiles = {}
        qengs = (nc.sync, nc.gpsimd)
        qi = 0
        HW_ = KC * PBP // 2
        for pb in range(NPP):
            for name, x_d, _, _ in streams:
                xin = pin.tile([128, KC * PBP], BF16, tag="xin")
                for hh in range(2):
                    qengs[qi % 2].dma_start(
                        xin[:, hh * HW_:(hh + 1) * HW_],
                        x_d[pb][:, hh * HW_:(hh + 1) * HW_],
                    )
                    qi += 1
                xin_tiles[(pb, name)] = xin

        # ---- projection work quanta ----
        def proj_quantum(pb, sname, half):
            xin = xin_tiles[(pb, sname)]
            _, _, dst, soff = next(s for s in streams if s[0] == sname)
            pt = psS.tile([128, 512], F32, tag="ps")
            c0 = half * 512
            for kc in range(KC):
                nc.tensor.matmul(
                    pt[:, 0:512],
                    w3_sb[:, soff * 1024 + kc * 128:soff * 1024 + (kc + 1) * 128],
                    xin[:, kc * PBP + c0:kc * PBP + c0 + 512],
                    start=(kc == 0), stop=(kc == KC - 1),
                )
            dslice = dst[:, pb * PBP + c0:pb * PBP + c0 + 512]
            if sname == "q":
                nc.vector.tensor_scalar_add(dslice, pt[:, 0:512], cf[:, 0:1])
            elif sname == "k":
                nc.vector.tensor_scalar_add(dslice, pt[:, 0:512], cf[:, 1:2])
            else:
                nc.vector.tensor_copy(dslice, pt[:, 0:512])

        def quanta_for_pb(pb):
            return [(lambda p=pb, s=s, h=h: proj_quantum(p, s, h))
                    for s in ("k", "v", "q") for h in (0, 1)]

        # ---- branch-2 dilation-compressed copies (per pb chunk) ----
        def b2_pair(pb, src, dst):
            for slot in range(2):
                p0 = 64 * slot
                o0 = 2 * slot
                dc = dst[p0:p0 + 64, pb * 256:(pb + 1) * 256]
                s0 = pb * PBP + o0
                nc.vector.tensor_scalar_mul(
                    dc,
                    src[p0:p0 + 64, s0:s0 + 4 * 255 + 1:4],
                    cf[p0:p0 + 64, 10:11],
                )
                nc.vector.scalar_tensor_tensor(
                    dc,
                    src[p0:p0 + 64, s0 + 1:s0 + 1 + 4 * 255 + 1:4],
                    cf[p0:p0 + 64, 11:12], dc,
                    mybir.AluOpType.mult, mybir.AluOpType.add,
                )

        def b2_v(pb):
            b2_pair(pb, VT, VT2)

        def b2_kq(pb):
            b2_pair(pb, KT, KT2)
            b2_pair(pb, QT, QT2)

        # ---- normalization: recip Z rows in place, broadcast via matmul ----
        def norm_recip(c0, w):
            with nc.allow_low_precision(reason="softmax denom reciprocal"):
                nc.vector.reciprocal(zzr[0:1, c0:c0 + w],
                                     accz0[64:65, c0:c0 + w])
                nc.vector.reciprocal(zzr[32:33, c0:c0 + w],
                                     accz1[64:65, c0:c0 + w])

        def norm_block35():
            # cols 3584..4096 (upper half of block 3)
            rb = psS.tile([128, 1024], F32, tag="ps")
            nc.tensor.matmul(rb[:, 0:512], ind_sb[:], zzr[0:33, 3584:4096],
                             start=True, stop=True)
            with nc.allow_low_precision(reason="bf16 a2a payload"):
                nc.vector.tensor_mul(accb[0:64, 3584:4096],
                                     accz0[0:64, 3584:4096], rb[0:64, 0:512])
                nc.vector.tensor_mul(accb[64:128, 3584:4096],
                                     accz1[0:64, 3584:4096], rb[64:128, 0:512])

        def norm_block(nb, w=1024):
            # rb = 1/Z broadcast to all 128 partitions via indicator matmul
            c0 = nb * 1024
            rb = psS.tile([128, 1024], F32, tag="ps")
            for hh in range(0, w, 512):
                nc.tensor.matmul(
                    rb[:, hh:hh + 512], ind_sb[:],
                    zzr[0:33, c0 + hh:c0 + hh + 512],
                    start=True, stop=True,
                )
            with nc.allow_low_precision(reason="bf16 a2a payload"):
                nc.vector.tensor_mul(
                    accb[0:64, c0:c0 + w],
                    accz0[0:64, c0:c0 + w], rb[0:64, 0:w])
                nc.vector.tensor_mul(
                    accb[64:128, c0:c0 + w],
                    accz1[0:64, c0:c0 + w], rb[64:128, 0:w])

        # ---- a2a staging / launch / gather (position-split chunks) ----
        # stage rides the DVE queue (fires right after the norm muls that
        # produce accb); gather rides the Pool queue (blocked during the
        # collective's flight, so it fires exactly at flight end). This
        # keeps the tile scheduler from interleaving them with outT DMAs.
        def a2a_stage(h, part=None):
            p0, cw = CHUNKS[h]
            r0, r1 = 0, 8
            if part is not None:
                r0, r1 = (0, 4) if part == 0 else (4, 8)
            nc.sync.dma_start(
                a2a_in[h][r0:r1].rearrange("r p c -> p r c"),
                accb[:, p0 + r0 * cw:p0 + r1 * cw].rearrange(
                    "p (r c) -> p r c", r=r1 - r0),
            )

        def a2a_launch(h, gather=True):
            nc.gpsimd.collective_compute(
                "AllToAll", mybir.AluOpType.bypass,
                replica_groups=[list(range(8))],
                ins=[a2a_in[h][:]], outs=[a2a_out[h][:]],
            )
            if gather:
                # chunk 1's gather rides ACT (idle then): on the Pool queue
                # it would sit between launch1 and launch2 and delay c2
                eng = nc.scalar if h == 1 else nc.gpsimd
                eng.dma_start(
                    mgr[h][:].rearrange("p (s c) -> p s c", s=8),
                    a2a_out[h][:].rearrange("s p c -> p s c"),
                )

        def a2a_gather_sp(h):
            # two halves: the first four shards land first so the output
            # projection's ec 0..3 matmuls can start during the second copy
            for s0, s1 in ((0, 4), (4, 8)):
                nc.sync.dma_start(
                    mgr[h][:].rearrange("p (s c) -> p s c", s=8)[:, s0:s1],
                    a2a_out[h][s0:s1].rearrange("s p c -> p s c"),
                )

        # outT column ranges per chunk
        OC0 = (0, 256, 384)

        # wide staging tiles for the tail chunks: all 8 ob blocks land
        # side by side so one DMA ships the whole chunk
        osb_all = [persist.tile([128, 8 * CHUNKS[h][1]], BF16, tag="osba",
                                name=f"osba{h}") for h in (1, 2)]

        # ---- output projection quantum: one 128-row output block ----
        def oq(h, ob):
            cw = CHUNKS[h][1]
            pt = psS.tile([128, 512], F32, tag="ps")
            for ec in range(KC):
                nc.tensor.matmul(
                    pt[:, 0:cw],
                    wo_sb[:, ec * E + ob * 128:ec * E + (ob + 1) * 128],
                    mgr[h][:, ec * cw:(ec + 1) * cw],
                    start=(ec == 0), stop=(ec == KC - 1),
                )
            if h == 0:
                osb = opool.tile([128, 256], BF16, tag="osb")
                # DVE during the last job (ACT is the per-job bottleneck)
                nc.vector.tensor_scalar_add(osb[:, 0:cw], pt[:, 0:cw],
                                            cf[:, 2 + ob:3 + ob])
                nc.sync.dma_start(
                    outT[ob * 128:(ob + 1) * 128, OC0[0]:OC0[0] + cw],
                    osb[:, 0:cw])
            else:
                # tail chunks: both ACT and DVE are idle — alternate so the
                # bias adds drain in parallel; ship once per chunk
                dst = osb_all[h - 1][:, ob * cw:(ob + 1) * cw]
                if ob % 2 == 0:
                    nc.scalar.activation(dst, pt[:, 0:cw], AF.Identity,
                                         bias=cf[:, 2 + ob:3 + ob])
                else:
                    nc.vector.tensor_scalar_add(dst, pt[:, 0:cw],
                                                cf[:, 2 + ob:3 + ob])

        def out_ship(h, half):
            cw = CHUNKS[h][1]
            o0, o1 = (0, 4) if half == 0 else (4, 8)
            nc.sync.dma_start(
                outT[o0 * 128:o1 * 128, OC0[h]:OC0[h] + cw].rearrange(
                    "(ob p) c -> p ob c", p=128),
                osb_all[h - 1][:, o0 * cw:o1 * cw].rearrange(
                    "p (ob c) -> p ob c", ob=o1 - o0))

        # ---- K/Q slicing per branch ----
        def kq_slice(br, seg, slot, t, lo, size):
            if br == 0:
                base = 1024 * seg + lo
                return t[slot * 64:(slot + 1) * 64, base:base + size]
            if br == 1:
                base = 2048 * seg + 2 * lo + slot
                return t[slot * 64:(slot + 1) * 64,
                         base:base + 2 * size - slot:2]
            return t[slot * 64:(slot + 1) * 64, lo:lo + size]

        # ---- V_both prep: PE transposes; copies ride ACT (idle at job
        # tails) so the merge-laden DVE queue never gates the next job
        def vb_prep(br, seg, vb=None, jcs=range(8)):
            if vb is None:
                vb = vpool.tile([128, 2 * 8 * VBW], BF16, tag="vb")
                nc.scalar.copy(vb[:, 64::VBW], ONES16)
            for jc in jcs:
                if br == 0:
                    tp = psS.tile([128, 128], BF16, tag="ps")
                    src = VT[:, 1024 * seg + 128 * jc:1024 * seg + 128 * (jc + 1)]
                    nc.tensor.transpose(tp[:, 0:128], src, cb[:, 128:256])
                    dst = vb[:].rearrange(
                        "p (s jj t) -> p s jj t", s=2, jj=8
                    )[:, :, jc, 0:64]
                    srcp = tp[:, 0:128].rearrange("p (s r) -> p s r", s=2)
                    nc.scalar.copy(dst, srcp)
                else:
                    for slot in range(2):
                        tp = psS.tile([128, 128], BF16, tag="ps")
                        if br == 1:
                            base = 2048 * seg + 256 * jc + slot
                            src = VT[slot * 64:(slot + 1) * 64,
                                     base:base + 256 - slot:2]
                        else:
                            src = VT2[slot * 64:(slot + 1) * 64,
                                      128 * jc:128 * (jc + 1)]
                        nc.tensor.transpose(tp[:, 0:64], src, eye_s(slot))
                        nc.scalar.copy(
                            vb[:, slot * 8 * VBW + jc * VBW:
                               slot * 8 * VBW + jc * VBW + 64],
                            tp[:, 0:64],
                        )
            return vb

        # ---- one (branch, segment) job ----
        def job(br, seg, fillers, vb=None, defer_final=False):
            kt_src = KT2 if br == 2 else KT
            qt_src = QT2 if br == 2 else QT
            fillers = list(fillers)
            if vb is None:
                vb = vb_prep(br, seg)

            o_ps_a = psO.tile([128, 1024], F32, tag="o")
            o_ps_b = psO.tile([128, 1024], F32, tag="o")
            o_ps = [o_ps_a, o_ps_b]

            def merge(r0, r1):
                # scatter o_ps[slot] rows [feats|Z] region [r0:r1] into accz
                for slot in range(2):
                    op = o_ps[slot]
                    az = accz[slot]
                    w = r1 - r0
                    if br == 0:
                        d0 = 1024 * seg + r0
                        dst = az[0:VBW, d0:d0 + w]
                        nc.vector.tensor_add(dst, dst, op[0:VBW, r0:r1])
                    elif br == 1:
                        d0 = 2048 * seg + 2 * r0 + slot
                        d1 = d0 + 2 * (w - 1) + 1
                        dst = az[0:VBW, d0:d1:2]
                        nc.vector.tensor_add(dst, dst, op[0:VBW, r0:r1])
                    else:
                        o0 = 2 * slot
                        for dd in range(2):
                            d0 = 4 * r0 + o0 + dd
                            d1 = d0 + 4 * (w - 1) + 1
                            dst = az[0:VBW, d0:d1:4]
                            nc.vector.scalar_tensor_tensor(
                                dst, op[0:VBW, r0:r1],
                                cf[0:VBW, 10 + dd:11 + dd],
                                dst, mybir.AluOpType.mult, mybir.AluOpType.add,
                            )

            def emit_avs(jc, es):
                c0 = 128 * jc
                for slot in range(2):
                    e = es[slot]
                    vbs = vb[:, slot * 8 * VBW + jc * VBW:
                             slot * 8 * VBW + (jc + 1) * VBW]
                    if c0 < 512:
                        nc.tensor.matmul(
                            o_ps[slot][0:VBW, c0:512], vbs, e[:, c0:512],
                            start=(jc == 0), stop=(jc == 3),
                        )
                        nc.tensor.matmul(
                            o_ps[slot][0:VBW, 512:1024], vbs, e[:, 512:1024],
                            start=(jc == 0), stop=(jc == 7),
                        )
                    else:
                        nc.tensor.matmul(
                            o_ps[slot][0:VBW, c0:1024], vbs, e[:, c0:1024],
                            start=(jc == 0), stop=(jc == 7),
                        )

            prev_es = None
            for jc in range(8):
                c0 = 128 * jc
                es = []
                for slot in range(2):
                    s = psS.tile([128, 1024], F32, tag="ps")
                    lhs = kq_slice(br, seg, slot, kt_src, c0, 128)
                    # causal-mask bias first: clears the bank (start=True),
                    # writes -100 upper-triangle into the diagonal block.
                    nc.tensor.matmul(
                        s[:, c0:c0 + 128], cb[:, 128:256], TRI,
                        start=True, stop=False,
                    )
                    if c0 < 512:
                        nc.tensor.matmul(
                            s[:, c0:512], lhs,
                            kq_slice(br, seg, slot, qt_src, c0, 512 - c0),
                            start=False, stop=True,
                            tile_position=(slot * 64, 0),
                        )
                        nc.tensor.matmul(
                            s[:, 512:1024], lhs,
                            kq_slice(br, seg, slot, qt_src, 512, 512),
                            start=True, stop=True,
                            tile_position=(slot * 64, 0),
                        )
                    else:
                        nc.tensor.matmul(
                            s[:, c0:1024], lhs,
                            kq_slice(br, seg, slot, qt_src, c0, 1024 - c0),
                            start=False, stop=True,
                            tile_position=(slot * 64, 0),
                        )
                    e = epool.tile([128, 1024], BF16, tag="e")
                    nc.scalar.activation(e[:, c0:1024], s[:, c0:1024], AF.Exp)
                    es.append(e)

                if fillers:
                    f = fillers.pop(0)
                    if f is not None:
                        f()

                if prev_es is not None:
                    emit_avs(jc - 1, prev_es)
                    if jc - 1 == 3:
                        merge(0, 512)
                prev_es = es
            emit_avs(7, prev_es)
            if defer_final:
                ret = lambda: merge(512, 1024)
            else:
                merge(512, 1024)
                ret = None
            for f in fillers:
                if f is not None:
                    f()
            return ret

        # ================= emission order =================
        # jobs ordered so positions 0..2048 finish (merge + norm) early:
        # chunk-0 a2a flies over jobs 5-6; chunk-1 (positions 2048..3072,
        # complete once br1 seg1's first-half merge lands) flies over job 7,
        # whose fillers run chunk-0's output projection; chunk-2 (3072..4096)
        # is the only exposed flight.
        for q in quanta_for_pb(0):
            q()
        b2_v(0)
        b2_kq(0)
        job(0, 0, quanta_for_pb(1) + [lambda: b2_v(1), lambda: b2_kq(1)])
        job(0, 1, quanta_for_pb(2) + [lambda: b2_v(2), lambda: b2_kq(2)])
        job(1, 0, quanta_for_pb(3) + [lambda: b2_v(3)])
        b2_kq(3)
        vbh = [None]

        def vbp(br, seg, jcs):
            def f():
                vbh[0] = vb_prep(br, seg, vb=vbh[0], jcs=jcs)
            return f

        m20 = job(2, 0, [None, None, None, None, None,
                         lambda: norm_recip(0, 2048),
                         vbp(0, 2, range(0, 4)), vbp(0, 2, range(4, 8))],
                  defer_final=True)
        vb02, vbh[0] = vbh[0], None
        job(0, 2, [lambda: (norm_block(0), a2a_stage(0, part=0)),
                   lambda: (norm_block(1), a2a_stage(0, part=1),
                            a2a_launch(0)),
                   m20, None, None, None, None, None], vb=vb02)
        m11 = job(1, 1, [None, None, None, None, None,
                         lambda: norm_recip(2048, 1024),
                         vbp(0, 3, range(0, 4)), vbp(0, 3, range(4, 8))],
                  defer_final=True)
        vb03, vbh[0] = vbh[0], None
        job(0, 3, [lambda: norm_block(2),
                   lambda: (a2a_stage(1), a2a_launch(1)), m11,
                   (lambda: oq(0, 0)), (lambda: oq(0, 1)),
                   (lambda: norm_recip(3072, 512)),
                   (lambda: (norm_block(3, 512), a2a_stage(2, part=0))),
                   (lambda: oq(0, 2))], vb=vb03)
        for b in range(3, 8):
            oq(0, b)

        # ---- last quarter tail: second 512, chunk-2 launch, projections ----
        norm_recip(3584, 512)
        norm_block35()
        a2a_stage(2, part=1)
        a2a_launch(2, gather=False)
        for b in range(8):
            oq(1, b)
        out_ship(1, 0)
        out_ship(1, 1)
        a2a_gather_sp(2)
        for b in range(8):
            oq(2, b)
        out_ship(2, 0)
        out_ship(2, 1)

    with tile.TileContext(nc) as tc, contextlib.ExitStack() as ctx:
        _emit(tc, ctx)

    nc.compile()
    return nc


_NC_CACHE = {}


def _get_nc():
    if "nc" not in _NC_CACHE:
        _NC_CACHE["nc"] = _build()
    return _NC_CACHE["nc"]


def _prep_inputs(query, key, value, Wq, bq, Wk, bk, Wv, bv, Wo, bo):
    """Host-side layout prep. Returns in_maps for the 8 cores."""
    def _tilein(x):
        # [pb, p, kc*1024] with 16KB-contiguous per-partition lines
        xT = np.ascontiguousarray(x[0].T).astype(BF)   # (E, L)
        xp = xT.reshape(KC, 128, NPP, PBP).transpose(2, 1, 0, 3)
        return np.ascontiguousarray(xp.reshape(NPP, 128, KC * PBP))

    qT = _tilein(query)
    kT = _tilein(key)
    vT = _tilein(value)

    WqT = np.ascontiguousarray(Wq.T) * np.float32(0.125)
    WkT = np.ascontiguousarray(Wk.T)
    WvT = np.ascontiguousarray(Wv.T)

    # permuted Wo.T rows to match a2a arriving-feature order
    perm = np.concatenate(
        [np.r_[64 * s:64 * s + 64, 512 + 64 * s:512 + 64 * s + 64]
         for s in range(8)]
    )
    WoT = np.ascontiguousarray(Wo.T)[perm]            # (E e', E o)
    wo_pack = np.zeros((128, 8 * E), np.float32)
    for ec in range(8):
        wo_pack[:, ec * E:(ec + 1) * E] = WoT[ec * 128:(ec + 1) * 128]
    wo_pack = wo_pack.astype(BF)

    bo_eff = (bo + bv @ Wo.T).astype(np.float32)
    bo8 = bo_eff.reshape(8, 128).T.copy()             # [p, ob]

    IND = np.zeros((33, 128), np.float32)
    IND[0, 0:64] = 1.0
    IND[32, 64:128] = 1.0
    # [k, q] layout: mask where q < k
    TRI = np.zeros((128, 128), np.float32)
    ki, qi = np.meshgrid(np.arange(128), np.arange(128), indexing="ij")
    TRI[qi < ki] = -100.0
    EYE = np.eye(128, dtype=np.float32)

    constb = np.zeros((128, 272), np.float32)
    constb[:, 0:128] = TRI
    constb[:, 128:256] = EYE
    constb[:, 256:272] = 1.0
    constb = constb.astype(BF)

    in_maps = []
    for c in range(8):
        fa = np.r_[64 * c:64 * c + 64]
        fb = np.r_[512 + 64 * c:512 + 64 * c + 64]
        sel = np.concatenate([fa, fb])
        w3 = np.zeros((128, 3 * KC * 128), np.float32)
        for soff, WT in ((0, WkT), (1, WvT), (2, WqT)):
            for kc in range(KC):
                w3[:, soff * 1024 + kc * 128:soff * 1024 + (kc + 1) * 128] = \
                    WT[kc * 128:(kc + 1) * 128][:, sel]
        constf = np.zeros((128, 12), np.float32)
        constf[:, 0] = bq[sel] * np.float32(0.125)
        constf[:, 1] = bk[sel]
        constf[:, 2:10] = bo8
        constf[:, 10 + c // 4] = 1.0   # branch-2 offset indicator
        in_maps.append({
            "qT": qT, "kT": kT, "vT": vT,
            "w3": w3.astype(BF),
            "wo": wo_pack,
            "constf": constf,
            "constb": constb,
            "ind2d": IND.astype(BF),
        })
    return in_maps


def kernel(query, key, value, Wq, bq, Wk, bk, Wv, bv, Wo, bo,
           _trace=False, _result_holder=None):
    args = [np.asarray(a, np.float32) for a in
            (query, key, value, Wq, bq, Wk, bk, Wv, bv, Wo, bo)]
    nc = _get_nc()
    in_maps = _prep_inputs(*args)
    res = bass_utils.run_bass_kernel_spmd(
        nc, in_maps, core_ids=list(range(N_CORES)), trace=_trace
    )
    if _result_holder is not None:
        _result_holder.append(res)
    # core c holds positions [256c, 256c+256) (cols 0:256),
    # [2048+128c, 2048+128c+128) (cols 256:384) and
    # [3072+128c, 3072+128c+128) (cols 384:512)
    outT = np.zeros((E, L), np.float32)
    for c in range(N_CORES):
        r = res.results[c]["outT"]
        outT[:, 256 * c:256 * (c + 1)] = r[:, 0:256]
        outT[:, 2048 + 128 * c:2048 + 128 * (c + 1)] = r[:, 256:384]
        outT[:, 3072 + 128 * c:3072 + 128 * (c + 1)] = r[:, 384:512]
    return np.ascontiguousarray(outT.T).reshape(1, L, E)
